# revision 20
# baseline (speedup 1.0000x reference)
"""Hybrid-sharded variant: 4 batch quarters x 2 sequence halves.

Per core: 1024 batches x 21 owned positions (+2 halo x slices, zero-fed at
global edges; half 1's 21st position is a discarded pad). Bytes/core:
x 23*1024*256 = 6.03MB + W 63 chunks = 2.06MB + out 5.51MB = 13.0MB
vs 14.5MB for pure batch-parallel -> ~2.8us less DMA at 360GB/s.

Uniform SPMD program: zero halo slices supply the window padding, so there
are no edge special cases on device; per-core differences live entirely in
host-side input assembly (which W/x slices + zeros each core gets).
"""

import os
import sys

import numpy as np
import ml_dtypes

for _p in ("/opt/trn_rl_repo", "/root/.axon_site/_ro/trn_rl_repo"):
    if os.path.isdir(_p) and _p not in sys.path:
        sys.path.append(_p)

from contextlib import ExitStack

import concourse.mybir as mybir
import concourse.tile as tile
from concourse import bacc
from concourse.bass_utils import run_bass_kernel_spmd
from concourse.tile import add_dep_helper

S = 41
F = 128
WIN = 3
N_CORES = 8
B_FULL = 4096

SH = 2                    # sequence halves
PO = 21                   # owned positions per half (half 1: 20 real + 1 pad)
XS = PO + 2               # x slices incl halo
BQ = 4                    # batch quarters
NB = B_FULL // BQ         # 1024 batches per core
NBT = NB // 128           # 8 batch sub-tiles
GS = 4                    # positions per PSUM bank group
XG = 4                    # x slices per load group

_nc_cache = {}


def _layout2():
    """Matmul groups over local positions 0..PO-1. Entry (i, jmin, ncons):
    stationary x-slice i (local; slice i = global h*PO-1+i), consumers
    positions j in [jmin, jmin+ncons) with tap w = i - j."""
    out = []
    for j0 in range(0, PO, GS):
        n = min(GS, PO - j0)
        ents = []
        for i in range(j0, j0 + n + 2):
            jmin = max(j0, i - 2)
            jmax = min(j0 + n - 1, i)
            if jmax >= jmin:
                ents.append((i, jmin, jmax - jmin + 1))
        out.append((j0, n, ents))
    return out


_LAYOUT = _layout2()
_WCOLS = [sum(nc_ * F for _, _, nc_ in ents) for _, _, ents in _LAYOUT]
_WTOT = sum(_WCOLS)

_STORE_GROUPS = []
for _g, (_s0, _npos, _ents) in enumerate(_LAYOUT):
    if _STORE_GROUPS and _npos < GS:
        _ps0, _pn, _gs = _STORE_GROUPS[-1]
        _STORE_GROUPS[-1] = (_ps0, _pn + _npos, _gs + [_g])
    else:
        _STORE_GROUPS.append((_s0, _npos, [_g]))

_NXG = (XS + XG - 1) // XG  # x load groups


def _build(has_bias: bool):
    bf16 = mybir.dt.bfloat16
    f32 = mybir.dt.float32
    f16 = mybir.dt.float16
    nc = bacc.Bacc("TRN2", target_bir_lowering=False, debug=False)
    xT = nc.dram_tensor("xT", [XS, F, NB], bf16, kind="ExternalInput").ap()
    Wg = nc.dram_tensor("Wg", [F, _WTOT], bf16, kind="ExternalInput").ap()
    bias = (
        nc.dram_tensor("bias", [1, PO * F], bf16, kind="ExternalInput").ap()
        if has_bias
        else None
    )
    out = nc.dram_tensor("out", [NB, PO, F], f16, kind="ExternalOutput").ap()

    with tile.TileContext(nc) as tc:
        with ExitStack() as ctx:
            xpool = ctx.enter_context(tc.tile_pool(name="xT", bufs=_NXG))
            wpool = ctx.enter_context(tc.tile_pool(name="W", bufs=len(_LAYOUT)))
            ppool = ctx.enter_context(tc.tile_pool(name="ps", bufs=4, space="PSUM"))
            opool = ctx.enter_context(
                tc.tile_pool(name="stage", bufs=len(_STORE_GROUPS))
            )

            xt, wt, load_insts = [], [], []
            wcol0 = 0
            for g in range(max(_NXG, len(_LAYOUT))):
                if g < _NXG:
                    ns = min(XG, XS - g * XG)
                    tx = xpool.tile([F, XG * NB], bf16)
                    eng = nc.sync if g == 0 else nc.scalar
                    li = eng.dma_start(
                        tx[:].rearrange("k (s b) -> k s b", b=NB)[:, :ns, :],
                        xT[g * XG : g * XG + ns].rearrange("s k b -> k s b"),
                    )
                    load_insts.append(li.ins)
                    xt.append(tx)
                if g < len(_LAYOUT):
                    tw = wpool.tile([F, max(_WCOLS)], bf16)
                    li = nc.scalar.dma_start(
                        tw[:, : _WCOLS[g]], Wg[:, wcol0 : wcol0 + _WCOLS[g]]
                    )
                    load_insts.append(li.ins)
                    wt.append(tw)
                    wcol0 += _WCOLS[g]
            store_gate = load_insts[-6]

            if has_bias:
                bpool = ctx.enter_context(tc.tile_pool(name="bias", bufs=1))
                bias_sb = bpool.tile([1, PO * F], bf16)
                nc.scalar.dma_start(bias_sb[:], bias[:])
                ones = bpool.tile([1, F], bf16)
                nc.vector.memset(ones[:], 1.0)

            out_r = out.rearrange("(t p) s f -> p t s f", p=128)

            for ss0, snpos, gids in _STORE_GROUPS:
                stage = opool.tile([128, NBT * snpos * F], f16, tag="stage")
                stage_c = stage[:].rearrange("p (t c) -> p t c", t=NBT)
                for g in gids:
                    s0, npos, ents = _LAYOUT[g]
                    # two batch-subtiles share one 2-bank PSUM tile so a
                    # single DVE relu covers both (DVE SEQ is tail-critical)
                    for btp in range(NBT // 2):
                        ps = ppool.tile([128, 2 * GS * F], f32)
                        for half in range(2):
                            bt = btp * 2 + half
                            hb = half * GS * F
                            n_mm = len(ents) + (1 if has_bias else 0)
                            wcol = 0
                            for j, (si, jmin, ncons) in enumerate(ents):
                                gi, sub = divmod(si, XG)
                                lhsT = xt[gi][
                                    :, sub * NB + bt * 128 : sub * NB + (bt + 1) * 128
                                ]
                                c0 = hb + (jmin - s0) * F
                                nc.tensor.matmul(
                                    ps[:, c0 : c0 + ncons * F],
                                    lhsT=lhsT,
                                    rhs=wt[g][:, wcol : wcol + ncons * F],
                                    start=(j == 0),
                                    stop=(j == n_mm - 1),
                                )
                                wcol += ncons * F
                            if has_bias:
                                nc.tensor.matmul(
                                    ps[:, hb : hb + npos * F],
                                    lhsT=ones[:],
                                    rhs=bias_sb[:, s0 * F : (s0 + npos) * F],
                                    start=False,
                                    stop=True,
                                )
                        d0 = (s0 - ss0) * F
                        nc.vector.tensor_scalar_max(
                            stage_c[:, btp * 2 : btp * 2 + 2, d0 : d0 + npos * F],
                            ps[:].rearrange("p (h c) -> p h c", h=2)[:, :, : npos * F],
                            0.0,
                        )
                stage_v = stage[:].rearrange("p (t s f) -> p t s f", t=NBT, f=F)
                # finer split on the big merged group so store quarters start
                # right behind each relu pair (it ends the kernel)
                nsplit = 4 if snpos > GS else 2
                for o in range(nsplit):
                    h = NBT // nsplit
                    st = nc.sync.dma_start(
                        out_r[:, o * h : (o + 1) * h, ss0 : ss0 + snpos, :],
                        stage_v[:, o * h : (o + 1) * h, :snpos, :],
                    )
                    add_dep_helper(
                        st.ins, store_gate, sync=True, reason="stores after loads"
                    )

    nc.compile()
    return nc


def _get_nc(has_bias: bool):
    if has_bias not in _nc_cache:
        _nc_cache[has_bias] = _build(has_bias)
    return _nc_cache[has_bias]


def _prep_in_maps(inputs: np.ndarray, W: np.ndarray, b: np.ndarray):
    has_bias = bool(np.any(b))
    xb = inputs.astype(ml_dtypes.bfloat16)
    Wb = W.astype(ml_dtypes.bfloat16)
    wgs, biases = [], []
    for h in range(SH):
        valid = min(PO, S - h * PO)
        Wh = np.zeros((PO, WIN * F, F), ml_dtypes.bfloat16)
        Wh[:valid] = Wb[h * PO : h * PO + valid]
        blocks = []
        for s0, npos, ents in _LAYOUT:
            for si, jmin, ncons in ents:
                for j in range(jmin, jmin + ncons):
                    w = si - j  # tap: slice si = position si-1 locally
                    blocks.append(Wh[j, w * F : (w + 1) * F, :])
        wgs.append(np.ascontiguousarray(np.concatenate(blocks, axis=1)))
        assert wgs[-1].shape == (F, _WTOT)
        if has_bias:
            bh = np.zeros((PO, F), ml_dtypes.bfloat16)
            bh[:valid] = b[h * PO : h * PO + valid].astype(ml_dtypes.bfloat16)
            biases.append(np.ascontiguousarray(bh.reshape(1, PO * F)))

    in_maps = []
    for c in range(N_CORES):
        h, bp = divmod(c, BQ)
        g0 = h * PO - 1  # global position of local x slice 0
        xs = np.zeros((XS, F, NB), ml_dtypes.bfloat16)
        glo, ghi = max(0, g0), min(S - 1, g0 + XS - 1)
        xs[glo - g0 : ghi - g0 + 1] = xb[
            bp * NB : (bp + 1) * NB, glo : ghi + 1, :
        ].transpose(1, 2, 0)
        m = {"xT": np.ascontiguousarray(xs), "Wg": wgs[h]}
        if has_bias:
            m["bias"] = biases[h]
        in_maps.append(m)
    return in_maps, has_bias


def kernel(inputs: np.ndarray, W: np.ndarray, b: np.ndarray) -> np.ndarray:
    inputs = np.asarray(inputs)
    W = np.asarray(W)
    b = np.asarray(b)
    assert inputs.shape == (B_FULL, S, F), inputs.shape
    in_maps, has_bias = _prep_in_maps(inputs, W, b)
    nc = _get_nc(has_bias)
    res = run_bass_kernel_spmd(nc, in_maps, list(range(N_CORES)))
    out = np.empty((B_FULL, S, F), np.float32)
    for c in range(N_CORES):
        h, bp = divmod(c, BQ)
        valid = min(PO, S - h * PO)
        out[bp * NB : (bp + 1) * NB, h * PO : h * PO + valid, :] = (
            res.results[c]["out"][:, :valid, :].astype(np.float32)
        )
    return out
